# revision 11
# baseline (speedup 1.0000x reference)
"""TRN2 Bass/Tile kernel: Bahdanau-attention GRU decoder step, 8-core SPMD.

Sharding:
  - Attention (the 17 GFLOP enc@W1 einsum) is data-parallel over batch
    (8 batches/core).
  - GRU is tensor-parallel over the hidden dim (128 units/core), computed in
    transposed [unit, batch] layout.
  - The 1024x32000 output projection is tensor-parallel over vocab
    (4000 cols/core).
  - Two tiny AllGathers stitch the stages: context^T (bf16, 16KB/rank) and
    state^T (bf16, 16KB/rank).

All heavy matmuls run in bf16 (1 cycle/row on the PE); accumulation is fp32
in PSUM. Biases are folded into appended weight rows on the host (the GRU
input bias rides the recurrent path's ones-row; V_b drops out of softmax).
"""
import numpy as np

import concourse.bacc as bacc
import concourse.bass as bass
import concourse.mybir as mybir
import concourse.tile as tile
from concourse.bass_utils import run_bass_kernel_spmd

N_CORES = 8
B, S, H, EMB, VOCAB = 64, 128, 1024, 512, 32000
BC = B // N_CORES        # batches per core
VSH = VOCAB // N_CORES   # vocab shard
HSH = H // N_CORES       # hidden shard
TOK = BC * S             # tokens per core (1024)
F32 = mybir.dt.float32
BF16 = mybir.dt.bfloat16
NPBF16 = mybir.dt.np(BF16)
AF = mybir.ActivationFunctionType
AX = mybir.AxisListType


def _build():
    nc = bacc.Bacc(None, num_devices=N_CORES)
    dd = nc.declare_dram_parameter

    # Per-core inputs (host-sharded / host-transposed / bias-folded).
    encT = dd("encT", [H, TOK], BF16, isOutput=False)        # [h, tok]
    encN = dd("encN", [TOK, H], BF16, isOutput=False)        # [tok, h]
    w1 = dd("w1", [H, H], BF16, isOutput=False)              # [h, k]
    w2p = dd("w2p", [H + 1, H], BF16, isOutput=False)        # [hin(+1), k]
    vkr = dd("vkr", [128, 8], BF16, isOutput=False)          # vkr[p,t]=V_k[t*128+p]
    htp = dd("htp", [H + 1, B], BF16, isOutput=False)        # hidden^T + ones row
    htl = dd("htl", [H + 1, BC], BF16, isOutput=False)       # local-batch slice of htp
    gk = dd("gk", [H + EMB + 1, 3 * HSH], BF16, isOutput=False)
    grk = dd("grk", [H + 1, 3 * HSH], BF16, isOutput=False)
    xeT = dd("xeT", [EMB, B], BF16, isOutput=False)          # x_emb^T
    hT = dd("hT", [HSH, B], F32, isOutput=False)             # hidden slice^T
    okp = dd("okp", [H, VSH], BF16, isOutput=False)          # out_k shard
    idbf = dd("idbf", [128, 128], BF16, isOutput=False)

    probs_sh = dd("probs_sh", [B, VSH], F32, isOutput=True)
    state_shT = dd("state_shT", [HSH, B], F32, isOutput=True)
    attn_sh = dd("attn_sh", [BC, S], F32, isOutput=True)

    HT8 = H // 128   # 8
    with tile.TileContext(nc) as tc:
        with (
            tc.tile_pool(name="sb", bufs=1) as sb,
            tc.tile_pool(name="sbw", bufs=1) as sbw,
            tc.tile_pool(name="dram", bufs=1, space="DRAM") as dram,
        ):
            # ---- input DMAs (emission order = DMA priority) ----
            encT_t = [sb.tile([128, TOK], BF16, name=f"encT{i}") for i in range(HT8)]
            w1_t = [sb.tile([128, H], BF16, name=f"w1_{i}") for i in range(HT8)]
            for i in range(HT8):
                for q in range(4):
                    r0, r1 = q * 32, (q + 1) * 32
                    nc.sync.dma_start(encT_t[i][r0:r1, :],
                                      encT[i * 128 + r0:i * 128 + r1, :])
                    nc.sync.dma_start(w1_t[i][r0:r1, :],
                                      w1[i * 128 + r0:i * 128 + r1, :])
            w2_t = [sb.tile([128 if i < HT8 else 1, H], BF16, name=f"w2_{i}")
                    for i in range(HT8 + 1)]
            for i in range(HT8):
                for q in range(2):
                    r0, r1 = q * 64, (q + 1) * 64
                    nc.sync.dma_start(w2_t[i][r0:r1, :],
                                      w2p[i * 128 + r0:i * 128 + r1, :])
            nc.sync.dma_start(w2_t[HT8][:], w2p[H:H + 1, :])
            htl_t = [sb.tile([128 if i < HT8 else 1, BC], BF16, name=f"htl{i}")
                     for i in range(HT8 + 1)]
            for i in range(HT8 + 1):
                nc.sync.dma_start(htl_t[i][:], htl[i * 128:min((i + 1) * 128, H + 1), :])
            vk_sb = sb.tile([128, 8], BF16)
            nc.sync.dma_start(vk_sb[:], vkr[:])
            id_sb = sb.tile([128, 128], BF16)
            nc.sync.dma_start(id_sb[:], idbf[:])
            encN_t = [sb.tile([128, H], BF16, name=f"encN{b}") for b in range(BC)]
            for b in range(BC):
                for q in range(2):
                    r0, r1 = q * 64, (q + 1) * 64
                    nc.sync.dma_start(encN_t[b][r0:r1, :],
                                      encN[b * S + r0:b * S + r1, :])
            htp_t = [sb.tile([128 if i < HT8 else 1, B], BF16, name=f"htp{i}")
                     for i in range(HT8 + 1)]
            for i in range(HT8 + 1):
                nc.sync.dma_start(htp_t[i][:], htp[i * 128:min((i + 1) * 128, H + 1), :])
            ones_row = htp_t[HT8]  # [1, 64] of 1.0 (host-built)
            GKT = (H + EMB) // 128  # 12
            gk_t = [sb.tile([128 if i < GKT else 1, 3 * HSH], BF16, name=f"gk{i}")
                    for i in range(GKT + 1)]
            for i in range(GKT + 1):
                nc.sync.dma_start(gk_t[i][:], gk[i * 128:min((i + 1) * 128, H + EMB + 1), :])
            grk_t = [sb.tile([128 if i < HT8 else 1, 3 * HSH], BF16, name=f"grk{i}")
                     for i in range(HT8 + 1)]
            for i in range(HT8 + 1):
                nc.sync.dma_start(grk_t[i][:], grk[i * 128:min((i + 1) * 128, H + 1), :])
            xe_t = [sb.tile([128, B], BF16, name=f"xe{i}") for i in range(EMB // 128)]
            for i in range(EMB // 128):
                nc.sync.dma_start(xe_t[i][:], xeT[i * 128:(i + 1) * 128, :])
            hT_sb = sb.tile([HSH, B], F32)
            nc.sync.dma_start(hT_sb[:], hT[:])
            # big output-projection weights last (prefetch, must not block the above)
            ok_t = [sbw.tile([128, VSH], BF16, name=f"ok{i}") for i in range(HT8)]
            for i in range(HT8):
                nc.sync.dma_start(ok_t[i][:], okp[i * 128:(i + 1) * 128, :])

            # collective bounce buffers
            cc1_in = dram.tile([BC, H], BF16)                 # context shard (b-major)
            cc1_out = dram.tile([B, H], BF16, addr_space="Shared")
            cc2_in = dram.tile([HSH, B], BF16)                # state^T shard
            cc2_out = dram.tile([N_CORES * HSH, B], BF16, addr_space="Shared")

            with (
                tc.tile_pool(name="pf", bufs=2, space="PSUM") as pf_pool,
                tc.tile_pool(name="psmall", bufs=2, space="PSUM") as psm_pool,
                tc.tile_pool(name="pscore", bufs=2, space="PSUM") as ps_pool,
            ):
                # ---- hW2^T[k, b_local] = W2p^T @ hiddenT(local) ----
                h2_sb = [sb.tile([128, BC], F32, name=f"h2_{k}") for k in range(HT8)]
                for kt in range(HT8):
                    ph2 = psm_pool.tile([128, BC], F32, tag="psm", name="ph2")
                    for i in range(HT8 + 1):
                        nc.tensor.matmul(
                            ph2[:], w2_t[i][:, kt * 128:(kt + 1) * 128], htl_t[i][:],
                            start=(i == 0), stop=(i == HT8))
                    nc.vector.tensor_copy(h2_sb[kt][:], ph2[:])

                # ---- feat^T = tanh(W1^T @ enc^T + hW2^T) ----
                featT_t = [sb.tile([128, TOK], BF16, name=f"featT{k}") for k in range(HT8)]
                for kt in range(HT8):
                    for ng in range(2):
                        pf = pf_pool.tile([128, 512], F32, tag="pf", name="pf")
                        for i in range(HT8):
                            nc.tensor.matmul(
                                pf[:], w1_t[i][:, kt * 128:(kt + 1) * 128],
                                encT_t[i][:, ng * 512:(ng + 1) * 512],
                                start=(i == 0), stop=(i == HT8 - 1))
                        for j in range(4):
                            b = ng * 4 + j
                            nc.scalar.activation(
                                featT_t[kt][:, b * 128:(b + 1) * 128],
                                pf[:, j * 128:(j + 1) * 128],
                                AF.Tanh, bias=h2_sb[kt][:, b:b + 1], scale=1.0)

                # ---- score[tok] = V^T @ feat^T ----
                score_row = sb.tile([1, TOK], F32)
                for ng in range(2):
                    ps = ps_pool.tile([1, 512], F32, tag="ps", name="ps")
                    for kt in range(HT8):
                        nc.tensor.matmul(
                            ps[:], vk_sb[:, kt:kt + 1],
                            featT_t[kt][:, ng * 512:(ng + 1) * 512],
                            start=(kt == 0), stop=(kt == HT8 - 1))
                    nc.vector.tensor_copy(score_row[:, ng * 512:(ng + 1) * 512], ps[:])

                # reshape [1, 1024] -> [8, 128] (SBUF->SBUF DMA)
                score_sb = sb.tile([BC, S], F32)
                nc.gpsimd.dma_start(score_sb[:], score_row[:])

                # ---- softmax over s per batch row ----
                negm = sb.tile([BC, 1], F32)
                nc.vector.reduce_max(negm[:], score_sb[:], axis=AX.X, negate=True)
                esum = sb.tile([BC, 1], F32)
                attn_e = sb.tile([BC, S], F32)
                nc.scalar.activation(attn_e[:], score_sb[:], AF.Exp,
                                     bias=negm[:], scale=1.0, accum_out=esum[:])
                rinv = sb.tile([BC, 1], F32)
                nc.vector.reciprocal(rinv[:], esum[:])
                attn_f = sb.tile([BC, S], F32)
                nc.vector.tensor_scalar_mul(attn_f[:], attn_e[:], rinv[:])
                nc.gpsimd.dma_start(attn_sh[:], attn_f[:])
                attn_bf = sb.tile([BC, S], BF16)
                nc.vector.tensor_scalar_mul(attn_bf[:], attn_e[:], rinv[:])

                # ---- attn^T [s, b] via PE transpose ----
                p_at = psm_pool.tile([S, BC], BF16, tag="psm", name="p_at")
                nc.tensor.transpose(p_at[:], attn_bf[:], id_sb[0:BC, 0:BC])
                attnT = sb.tile([S, BC], BF16)
                nc.vector.tensor_copy(attnT[:], p_at[:])

                # ---- context^T[h, b] = enc[b]^T @ attn[b], then PE-transpose
                # back to b-major so the AllGather buffer is contiguous ----
                ctxN_sb = sb.tile([BC, H], BF16)
                for i in range(HT8):
                    pc = psm_pool.tile([128, BC], F32, tag="psm", name="pc")
                    for b in range(BC):
                        nc.tensor.matmul(
                            pc[:, b:b + 1], encN_t[b][:, i * 128:(i + 1) * 128],
                            attnT[:, b:b + 1], start=True, stop=True)
                    ctxT_sb = sb.tile([128, BC], BF16, tag="ctxT", bufs=2,
                                      name="ctxT_sb")
                    nc.vector.tensor_copy(ctxT_sb[:], pc[:])
                    pcn = psm_pool.tile([BC, 128], BF16, tag="pcn", name="pcn")
                    nc.tensor.transpose(pcn[:], ctxT_sb[:], id_sb[:])
                    nc.vector.tensor_copy(ctxN_sb[:, i * 128:(i + 1) * 128], pcn[:])
                nc.sync.dma_start(cc1_in[:], ctxN_sb[:])

            # ---- AllGather context^T ----
            nc.gpsimd.collective_compute(
                "AllGather", mybir.AluOpType.bypass,
                replica_groups=[list(range(N_CORES))],
                ins=[cc1_in.opt()], outs=[cc1_out.opt()])

            # xT tiles: [h, b_global] — gather columns across cores
            ctxg_sb = sb.tile([B, H], BF16)
            for q in range(4):
                r0, r1 = q * 16, (q + 1) * 16
                eng = nc.sync if q % 2 == 0 else nc.scalar
                eng.dma_start(ctxg_sb[r0:r1, :], cc1_out[r0:r1, :])

            with (
                tc.tile_pool(name="pg", bufs=1, space="PSUM") as pg_pool,
                tc.tile_pool(name="pout", bufs=2, space="PSUM") as po_pool,
            ):
                xT_t = [sb.tile([128, B], BF16, name=f"xT{i}") for i in range(HT8)]
                for i in range(HT8):
                    ptr = po_pool.tile([128, B], BF16, tag="po", name="ptr")
                    nc.tensor.transpose(ptr[:], ctxg_sb[:, i * 128:(i + 1) * 128],
                                        id_sb[0:B, 0:B])
                    nc.vector.tensor_copy(xT_t[i][:], ptr[:])
                # ---- GRU in transposed layout: [unit, b] ----
                phm = [pg_pool.tile([HSH, B], F32, tag=f"phm{g}", name=f"phm{g}") for g in range(3)]
                for g in range(3):
                    for i in range(HT8 + 1):
                        nc.tensor.matmul(
                            phm[g][:], grk_t[i][:, g * HSH:(g + 1) * HSH], htp_t[i][:],
                            start=(i == 0), stop=(i == HT8))
                pxm = [pg_pool.tile([HSH, B], F32, tag=f"pxm{g}", name=f"pxm{g}") for g in range(3)]
                for g in range(3):
                    for i in range(GKT + 1):
                        rhs = xT_t[i] if i < HT8 else (
                            xe_t[i - HT8] if i < GKT else ones_row)
                        nc.tensor.matmul(
                            pxm[g][:], gk_t[i][:, g * HSH:(g + 1) * HSH], rhs[:],
                            start=(i == 0), stop=(i == GKT))

                hm_sb = [sb.tile([HSH, B], F32, name=f"hm{g}") for g in range(3)]
                for g in range(3):
                    nc.vector.tensor_copy(hm_sb[g][:], phm[g][:])
                zpre = sb.tile([HSH, B], F32)
                nc.vector.tensor_add(zpre[:], pxm[0][:], hm_sb[0][:])
                z_sb = sb.tile([HSH, B], F32)
                nc.scalar.activation(z_sb[:], zpre[:], AF.Sigmoid)
                rpre = sb.tile([HSH, B], F32)
                nc.vector.tensor_add(rpre[:], pxm[1][:], hm_sb[1][:])
                r_sb = sb.tile([HSH, B], F32)
                nc.scalar.activation(r_sb[:], rpre[:], AF.Sigmoid)
                rhh = sb.tile([HSH, B], F32)
                nc.vector.tensor_mul(rhh[:], r_sb[:], hm_sb[2][:])
                hpre = sb.tile([HSH, B], F32)
                nc.vector.tensor_add(hpre[:], pxm[2][:], rhh[:])
                hc_sb = sb.tile([HSH, B], F32)
                nc.scalar.activation(hc_sb[:], hpre[:], AF.Tanh)
                # state = hc + z*(h_old - hc)
                dd_sb = sb.tile([HSH, B], F32)
                nc.vector.tensor_sub(dd_sb[:], hT_sb[:], hc_sb[:])
                zd_sb = sb.tile([HSH, B], F32)
                nc.vector.tensor_mul(zd_sb[:], z_sb[:], dd_sb[:])
                stT_f = sb.tile([HSH, B], F32)
                nc.vector.tensor_add(stT_f[:], hc_sb[:], zd_sb[:])
                nc.gpsimd.dma_start(state_shT[:], stT_f[:])
                stT_bf_loc = sb.tile([HSH, B], BF16)
                nc.vector.tensor_copy(stT_bf_loc[:], stT_f[:])
                nc.gpsimd.dma_start(cc2_in[:], stT_bf_loc[:])

                # ---- AllGather state^T ----
                nc.gpsimd.collective_compute(
                    "AllGather", mybir.AluOpType.bypass,
                    replica_groups=[list(range(N_CORES))],
                    ins=[cc2_in.opt()], outs=[cc2_out.opt()])
                stT_t = [sb.tile([128, B], BF16, name=f"stT{i}") for i in range(HT8)]
                for i in range(HT8):
                    eng = nc.sync if i % 2 == 0 else nc.scalar
                    eng.dma_start(stT_t[i][:], cc2_out[i * 128:(i + 1) * 128, :])

                # ---- probs shard = state @ out_k(+bias row) ----
                NG = VSH // 500  # 8 groups of 500
                for ng in range(NG):
                    po = po_pool.tile([128, 500], F32, tag="po", name="po")
                    for i in range(HT8):
                        half = i % 2
                        nc.tensor.matmul(
                            po[half * B:(half + 1) * B, :], stT_t[i][:],
                            ok_t[i][:, ng * 500:(ng + 1) * 500],
                            start=(i < 2), stop=(i >= HT8 - 2),
                            tile_position=(0, half * B))
                    podd = sb.tile([B, 500], F32, tag="podd", bufs=3, name="podd")
                    nc.scalar.copy(podd[:], po[B:2 * B, :])
                    pr_sb = sb.tile([B, 500], F32, tag="prout", bufs=3, name="pr_sb")
                    nc.vector.tensor_add(pr_sb[:], po[0:B, :], podd[:])
                    nc.sync.dma_start(probs_sh[:, ng * 500:(ng + 1) * 500], pr_sb[:])
    nc.compile()
    return nc


_CACHE: dict = {}


def _get_nc():
    if "nc" not in _CACHE:
        _CACHE["nc"] = _build()
    return _CACHE["nc"]


def _prep_in_maps(inputs):
    f32 = np.float32
    dec = np.asarray(inputs["dec_input"])
    hid = np.asarray(inputs["hidden_state"], f32)
    enc = np.asarray(inputs["enc_output"], f32)
    emb = np.asarray(inputs["emb"], f32)
    W1 = np.asarray(inputs["W1_k"], f32)
    W1b = np.asarray(inputs["W1_b"], f32)
    W2 = np.asarray(inputs["W2_k"], f32)
    W2b = np.asarray(inputs["W2_b"], f32)
    Vk = np.asarray(inputs["V_k"], f32)
    gkf = np.asarray(inputs["gru_k"], f32)
    grkf = np.asarray(inputs["gru_rk"], f32)
    gb = np.asarray(inputs["gru_b"], f32)
    ok = np.asarray(inputs["out_k"], f32)
    ob = np.asarray(inputs["out_b"], f32)

    xemb = emb[np.asarray(dec[:, 0], dtype=np.int64)]          # (B, EMB)

    def bf(a):
        return np.ascontiguousarray(np.asarray(a, dtype=f32), dtype=NPBF16)

    def fc(a):
        return np.ascontiguousarray(a, dtype=f32)

    w2p = np.vstack([W2, (W1b + W2b)[None, :]])
    htp = np.vstack([hid.T, np.ones((1, B), f32)])
    vkr = Vk.reshape(H // 128, 128).T
    gkb = np.vstack([gkf, gb[0][None, :]])
    grkb = np.vstack([grkf, gb[1][None, :]])
    okp = ok
    ident = np.eye(128, dtype=NPBF16)

    htp_bf = bf(htp)
    vkr_bf = bf(vkr)
    w1_bf = bf(W1)
    w2p_bf = bf(w2p)
    xeT_bf = bf(xemb.T)

    maps = []
    for c in range(N_CORES):
        bs = slice(c * BC, (c + 1) * BC)
        hs = slice(c * HSH, (c + 1) * HSH)
        cols = np.concatenate([
            np.arange(c * HSH, (c + 1) * HSH),
            np.arange(H + c * HSH, H + (c + 1) * HSH),
            np.arange(2 * H + c * HSH, 2 * H + (c + 1) * HSH),
        ])
        encN_ = enc[bs].reshape(TOK, H)
        maps.append({
            "encT": bf(encN_.T),
            "encN": bf(encN_),
            "w1": w1_bf,
            "w2p": w2p_bf,
            "vkr": vkr_bf,
            "htp": htp_bf,
            "htl": np.ascontiguousarray(htp_bf[:, bs]),
            "gk": bf(gkb[:, cols]),
            "grk": bf(grkb[:, cols]),
            "xeT": xeT_bf,
            "hT": fc(hid[:, hs].T),
            "okp": bf(okp[:, c * VSH:(c + 1) * VSH]),
            "idbf": ident,
        })
    return maps


def _assemble(results, ob):
    probs = np.concatenate([results[c]["probs_sh"] for c in range(N_CORES)], axis=1)
    probs += ob[None, :]
    state = np.concatenate(
        [results[c]["state_shT"].T for c in range(N_CORES)], axis=1)
    attn = np.concatenate(
        [results[c]["attn_sh"] for c in range(N_CORES)], axis=0)[:, :, None]
    return (np.ascontiguousarray(probs), np.ascontiguousarray(state),
            np.ascontiguousarray(attn))


def run(inputs, trace=False, tmpdir=None, trace_cores=None):
    nc = _get_nc()
    in_maps = _prep_in_maps(inputs)
    res = run_bass_kernel_spmd(nc, in_maps, list(range(N_CORES)),
                               trace=trace, tmpdir=tmpdir,
                               trace_cores=trace_cores)
    ob = np.asarray(inputs["out_b"], np.float32)
    return _assemble(res.results, ob), res


def kernel(**inputs):
    (probs, state, attn), _ = run(inputs, trace=False)
    return probs, state, attn


# revision 12
# speedup vs baseline: 1.0098x; 1.0098x over previous
"""TRN2 Bass/Tile kernel: Bahdanau-attention GRU decoder step, 8-core SPMD.

Sharding:
  - Attention (the 17 GFLOP enc@W1 einsum) is data-parallel over batch
    (8 batches/core).
  - GRU is tensor-parallel over the hidden dim (128 units/core), computed in
    transposed [unit, batch] layout.
  - The 1024x32000 output projection is tensor-parallel over vocab
    (4000 cols/core).
  - Two tiny AllGathers stitch the stages: context^T (bf16, 16KB/rank) and
    state^T (bf16, 16KB/rank).

All heavy matmuls run in bf16 (1 cycle/row on the PE); accumulation is fp32
in PSUM. Biases are folded into appended weight rows on the host (the GRU
input bias rides the recurrent path's ones-row; V_b drops out of softmax).
"""
import numpy as np

import concourse.bacc as bacc
import concourse.bass as bass
import concourse.mybir as mybir
import concourse.tile as tile
from concourse.bass_utils import run_bass_kernel_spmd

N_CORES = 8
B, S, H, EMB, VOCAB = 64, 128, 1024, 512, 32000
BC = B // N_CORES        # batches per core
VSH = VOCAB // N_CORES   # vocab shard
HSH = H // N_CORES       # hidden shard
TOK = BC * S             # tokens per core (1024)
F32 = mybir.dt.float32
BF16 = mybir.dt.bfloat16
NPBF16 = mybir.dt.np(BF16)
AF = mybir.ActivationFunctionType
AX = mybir.AxisListType


def _build():
    nc = bacc.Bacc(None, num_devices=N_CORES)
    dd = nc.declare_dram_parameter

    # Per-core inputs (host-sharded / host-transposed / bias-folded).
    encT = dd("encT", [H, TOK], BF16, isOutput=False)        # [h, tok]
    encN = dd("encN", [TOK, H], BF16, isOutput=False)        # [tok, h]
    w1 = dd("w1", [H, H], BF16, isOutput=False)              # [h, k]
    w2p = dd("w2p", [H + 1, H], BF16, isOutput=False)        # [hin(+1), k]
    vkr = dd("vkr", [128, 8], BF16, isOutput=False)          # vkr[p,t]=V_k[t*128+p]
    htp = dd("htp", [H + 1, B], BF16, isOutput=False)        # hidden^T + ones row
    htl = dd("htl", [H + 1, BC], BF16, isOutput=False)       # local-batch slice of htp
    gk = dd("gk", [H + EMB + 1, 3 * HSH], BF16, isOutput=False)
    grk = dd("grk", [H + 1, 3 * HSH], BF16, isOutput=False)
    xeT = dd("xeT", [EMB, B], BF16, isOutput=False)          # x_emb^T
    hT = dd("hT", [HSH, B], F32, isOutput=False)             # hidden slice^T
    okp = dd("okp", [H, VSH], BF16, isOutput=False)          # out_k shard
    idbf = dd("idbf", [128, 128], BF16, isOutput=False)

    probs_sh = dd("probs_sh", [B, VSH], F32, isOutput=True)
    state_shT = dd("state_shT", [HSH, B], F32, isOutput=True)
    attn_sh = dd("attn_sh", [BC, S], F32, isOutput=True)

    HT8 = H // 128   # 8
    with tile.TileContext(nc) as tc:
        rings = [nc.sync, nc.scalar]
        with (
            tc.tile_pool(name="sb", bufs=1) as sb,
            tc.tile_pool(name="sbw", bufs=1) as sbw,
            tc.tile_pool(name="dram", bufs=1, space="DRAM") as dram,
        ):
            # ---- input DMAs (emission order = DMA priority) ----
            encT_t = [sb.tile([128, TOK], BF16, name=f"encT{i}") for i in range(HT8)]
            w1_t = [sb.tile([128, H], BF16, name=f"w1_{i}") for i in range(HT8)]
            for i in range(HT8):
                for q in range(2):
                    r0, r1 = q * 64, (q + 1) * 64
                    rings[q].dma_start(encT_t[i][r0:r1, :],
                                       encT[i * 128 + r0:i * 128 + r1, :])
                    rings[1 - q].dma_start(w1_t[i][r0:r1, :],
                                           w1[i * 128 + r0:i * 128 + r1, :])
            w2_t = [sb.tile([128 if i < HT8 else 1, H], BF16, name=f"w2_{i}")
                    for i in range(HT8 + 1)]
            for i in range(HT8):
                for q in range(2):
                    r0, r1 = q * 64, (q + 1) * 64
                    rings[(i + q) % 2].dma_start(w2_t[i][r0:r1, :],
                                                 w2p[i * 128 + r0:i * 128 + r1, :])
            nc.sync.dma_start(w2_t[HT8][:], w2p[H:H + 1, :])
            htl_t = [sb.tile([128 if i < HT8 else 1, BC], BF16, name=f"htl{i}")
                     for i in range(HT8 + 1)]
            for i in range(HT8 + 1):
                rings[i % 2].dma_start(htl_t[i][:],
                                       htl[i * 128:min((i + 1) * 128, H + 1), :])
            vk_sb = sb.tile([128, 8], BF16)
            nc.sync.dma_start(vk_sb[:], vkr[:])
            id_sb = sb.tile([128, 128], BF16)
            nc.sync.dma_start(id_sb[:], idbf[:])
            encN_t = [sb.tile([128, H], BF16, name=f"encN{b}") for b in range(BC)]
            for b in range(BC):
                rings[b % 2].dma_start(encN_t[b][:], encN[b * S:(b + 1) * S, :])
            htp_t = [sb.tile([128 if i < HT8 else 1, B], BF16, name=f"htp{i}")
                     for i in range(HT8 + 1)]
            for i in range(HT8 + 1):
                rings[i % 2].dma_start(htp_t[i][:],
                                       htp[i * 128:min((i + 1) * 128, H + 1), :])
            ones_row = htp_t[HT8]  # [1, 64] of 1.0 (host-built)
            GKT = (H + EMB) // 128  # 12
            gk_t = [sb.tile([128 if i < GKT else 1, 3 * HSH], BF16, name=f"gk{i}")
                    for i in range(GKT + 1)]
            for i in range(GKT + 1):
                rings[i % 2].dma_start(gk_t[i][:],
                                       gk[i * 128:min((i + 1) * 128, H + EMB + 1), :])
            grk_t = [sb.tile([128 if i < HT8 else 1, 3 * HSH], BF16, name=f"grk{i}")
                     for i in range(HT8 + 1)]
            for i in range(HT8 + 1):
                rings[(i + 1) % 2].dma_start(grk_t[i][:],
                                             grk[i * 128:min((i + 1) * 128, H + 1), :])
            xe_t = [sb.tile([128, B], BF16, name=f"xe{i}") for i in range(EMB // 128)]
            for i in range(EMB // 128):
                rings[i % 2].dma_start(xe_t[i][:], xeT[i * 128:(i + 1) * 128, :])
            hT_sb = sb.tile([HSH, B], F32)
            nc.sync.dma_start(hT_sb[:], hT[:])
            # big output-projection weights last (prefetch, must not block the above)
            ok_t = [sbw.tile([128, VSH], BF16, name=f"ok{i}") for i in range(HT8)]
            for i in range(HT8):
                rings[i % 2].dma_start(ok_t[i][:], okp[i * 128:(i + 1) * 128, :])

            # collective bounce buffers
            cc1_in = dram.tile([BC, H], BF16)                 # context shard (b-major)
            cc1_out = dram.tile([B, H], BF16, addr_space="Shared")
            cc2_in = dram.tile([HSH, B], BF16)                # state^T shard
            cc2_out = dram.tile([N_CORES * HSH, B], BF16, addr_space="Shared")

            with (
                tc.tile_pool(name="pf", bufs=3, space="PSUM") as pf_pool,
                tc.tile_pool(name="psmall", bufs=2, space="PSUM") as psm_pool,
                tc.tile_pool(name="pscore", bufs=1, space="PSUM") as ps_pool,
            ):
                # ---- hW2^T[k, b_local] = W2p^T @ hiddenT(local) ----
                h2_sb = [sb.tile([128, BC], F32, name=f"h2_{k}") for k in range(HT8)]
                for kt in range(HT8):
                    ph2 = psm_pool.tile([128, BC], F32, tag="psm", name="ph2")
                    for i in range(HT8 + 1):
                        nc.tensor.matmul(
                            ph2[:], w2_t[i][:, kt * 128:(kt + 1) * 128], htl_t[i][:],
                            start=(i == 0), stop=(i == HT8))
                    nc.vector.tensor_copy(h2_sb[kt][:], ph2[:])

                # ---- feat^T = tanh(W1^T @ enc^T + hW2^T) ----
                featT_t = [sb.tile([128, TOK], BF16, name=f"featT{k}") for k in range(HT8)]
                for kt in range(HT8):
                    for ng in range(2):
                        pf = pf_pool.tile([128, 512], F32, tag="pf", name="pf")
                        for i in range(HT8):
                            nc.tensor.matmul(
                                pf[:], w1_t[i][:, kt * 128:(kt + 1) * 128],
                                encT_t[i][:, ng * 512:(ng + 1) * 512],
                                start=(i == 0), stop=(i == HT8 - 1))
                        for j in range(4):
                            b = ng * 4 + j
                            nc.scalar.activation(
                                featT_t[kt][:, b * 128:(b + 1) * 128],
                                pf[:, j * 128:(j + 1) * 128],
                                AF.Tanh, bias=h2_sb[kt][:, b:b + 1], scale=1.0)

                # ---- score[tok] = V^T @ feat^T ----
                score_row = sb.tile([1, TOK], F32)
                for ng in range(2):
                    ps = ps_pool.tile([1, 512], F32, tag="ps", name="ps")
                    for kt in range(HT8):
                        nc.tensor.matmul(
                            ps[:], vk_sb[:, kt:kt + 1],
                            featT_t[kt][:, ng * 512:(ng + 1) * 512],
                            start=(kt == 0), stop=(kt == HT8 - 1))
                    nc.vector.tensor_copy(score_row[:, ng * 512:(ng + 1) * 512], ps[:])

                # reshape [1, 1024] -> [8, 128] (SBUF->SBUF DMA)
                score_sb = sb.tile([BC, S], F32)
                nc.gpsimd.dma_start(score_sb[:], score_row[:])

                # ---- softmax over s per batch row ----
                negm = sb.tile([BC, 1], F32)
                nc.vector.reduce_max(negm[:], score_sb[:], axis=AX.X, negate=True)
                esum = sb.tile([BC, 1], F32)
                attn_e = sb.tile([BC, S], F32)
                nc.scalar.activation(attn_e[:], score_sb[:], AF.Exp,
                                     bias=negm[:], scale=1.0, accum_out=esum[:])
                rinv = sb.tile([BC, 1], F32)
                nc.vector.reciprocal(rinv[:], esum[:])
                attn_f = sb.tile([BC, S], F32)
                nc.vector.tensor_scalar_mul(attn_f[:], attn_e[:], rinv[:])
                nc.gpsimd.dma_start(attn_sh[:], attn_f[:])
                attn_bf = sb.tile([BC, S], BF16)
                nc.vector.tensor_scalar_mul(attn_bf[:], attn_e[:], rinv[:])

                # ---- attn^T [s, b] via PE transpose ----
                p_at = psm_pool.tile([S, BC], BF16, tag="psm", name="p_at")
                nc.tensor.transpose(p_at[:], attn_bf[:], id_sb[0:BC, 0:BC])
                attnT = sb.tile([S, BC], BF16)
                nc.vector.tensor_copy(attnT[:], p_at[:])

                # ---- context^T[h, b] = enc[b]^T @ attn[b], then PE-transpose
                # back to b-major so the AllGather buffer is contiguous ----
                ctxN_sb = sb.tile([BC, H], BF16)
                for i in range(HT8):
                    pc = psm_pool.tile([128, BC], F32, tag="psm", name="pc")
                    for b in range(BC):
                        nc.tensor.matmul(
                            pc[:, b:b + 1], encN_t[b][:, i * 128:(i + 1) * 128],
                            attnT[:, b:b + 1], start=True, stop=True)
                    ctxT_sb = sb.tile([128, BC], BF16, tag="ctxT", bufs=2,
                                      name="ctxT_sb")
                    nc.vector.tensor_copy(ctxT_sb[:], pc[:])
                    pcn = psm_pool.tile([BC, 128], BF16, tag="pcn", name="pcn")
                    nc.tensor.transpose(pcn[:], ctxT_sb[:], id_sb[:])
                    nc.vector.tensor_copy(ctxN_sb[:, i * 128:(i + 1) * 128], pcn[:])
                nc.sync.dma_start(cc1_in[:], ctxN_sb[:])

            # ---- AllGather context^T ----
            nc.gpsimd.collective_compute(
                "AllGather", mybir.AluOpType.bypass,
                replica_groups=[list(range(N_CORES))],
                ins=[cc1_in.opt()], outs=[cc1_out.opt()])

            # xT tiles: [h, b_global] — gather columns across cores
            ctxg_sb = sb.tile([B, H], BF16)
            for q in range(4):
                r0, r1 = q * 16, (q + 1) * 16
                eng = nc.sync if q % 2 == 0 else nc.scalar
                eng.dma_start(ctxg_sb[r0:r1, :], cc1_out[r0:r1, :])

            with (
                tc.tile_pool(name="pg", bufs=1, space="PSUM") as pg_pool,
                tc.tile_pool(name="pout", bufs=2, space="PSUM") as po_pool,
            ):
                xT_t = [sb.tile([128, B], BF16, name=f"xT{i}") for i in range(HT8)]
                for i in range(HT8):
                    ptr = po_pool.tile([128, B], BF16, tag="po", name="ptr")
                    nc.tensor.transpose(ptr[:], ctxg_sb[:, i * 128:(i + 1) * 128],
                                        id_sb[0:B, 0:B])
                    nc.vector.tensor_copy(xT_t[i][:], ptr[:])
                # ---- GRU in transposed layout: [unit, b] ----
                phm = [pg_pool.tile([HSH, B], F32, tag=f"phm{g}", name=f"phm{g}") for g in range(3)]
                for g in range(3):
                    for i in range(HT8 + 1):
                        nc.tensor.matmul(
                            phm[g][:], grk_t[i][:, g * HSH:(g + 1) * HSH], htp_t[i][:],
                            start=(i == 0), stop=(i == HT8))
                pxm = [pg_pool.tile([HSH, B], F32, tag=f"pxm{g}", name=f"pxm{g}") for g in range(3)]
                for g in range(3):
                    for i in range(GKT + 1):
                        rhs = xT_t[i] if i < HT8 else (
                            xe_t[i - HT8] if i < GKT else ones_row)
                        nc.tensor.matmul(
                            pxm[g][:], gk_t[i][:, g * HSH:(g + 1) * HSH], rhs[:],
                            start=(i == 0), stop=(i == GKT))

                hm_sb = [sb.tile([HSH, B], F32, name=f"hm{g}") for g in range(3)]
                for g in range(3):
                    nc.vector.tensor_copy(hm_sb[g][:], phm[g][:])
                zpre = sb.tile([HSH, B], F32)
                nc.vector.tensor_add(zpre[:], pxm[0][:], hm_sb[0][:])
                z_sb = sb.tile([HSH, B], F32)
                nc.scalar.activation(z_sb[:], zpre[:], AF.Sigmoid)
                rpre = sb.tile([HSH, B], F32)
                nc.vector.tensor_add(rpre[:], pxm[1][:], hm_sb[1][:])
                r_sb = sb.tile([HSH, B], F32)
                nc.scalar.activation(r_sb[:], rpre[:], AF.Sigmoid)
                rhh = sb.tile([HSH, B], F32)
                nc.vector.tensor_mul(rhh[:], r_sb[:], hm_sb[2][:])
                hpre = sb.tile([HSH, B], F32)
                nc.vector.tensor_add(hpre[:], pxm[2][:], rhh[:])
                hc_sb = sb.tile([HSH, B], F32)
                nc.scalar.activation(hc_sb[:], hpre[:], AF.Tanh)
                # state = hc + z*(h_old - hc)
                dd_sb = sb.tile([HSH, B], F32)
                nc.vector.tensor_sub(dd_sb[:], hT_sb[:], hc_sb[:])
                zd_sb = sb.tile([HSH, B], F32)
                nc.vector.tensor_mul(zd_sb[:], z_sb[:], dd_sb[:])
                stT_f = sb.tile([HSH, B], F32)
                nc.vector.tensor_add(stT_f[:], hc_sb[:], zd_sb[:])
                nc.gpsimd.dma_start(state_shT[:], stT_f[:])
                stT_bf_loc = sb.tile([HSH, B], BF16)
                nc.vector.tensor_copy(stT_bf_loc[:], stT_f[:])
                nc.gpsimd.dma_start(cc2_in[:], stT_bf_loc[:])

                # ---- AllGather state^T ----
                nc.gpsimd.collective_compute(
                    "AllGather", mybir.AluOpType.bypass,
                    replica_groups=[list(range(N_CORES))],
                    ins=[cc2_in.opt()], outs=[cc2_out.opt()])
                stT_t = [sb.tile([128, B], BF16, name=f"stT{i}") for i in range(HT8)]
                for i in range(HT8):
                    eng = nc.sync if i % 2 == 0 else nc.scalar
                    eng.dma_start(stT_t[i][:], cc2_out[i * 128:(i + 1) * 128, :])

                # ---- probs shard = state @ out_k(+bias row) ----
                NG = VSH // 500  # 8 groups of 500
                for ng in range(NG):
                    po = po_pool.tile([128, 500], F32, tag="po", name="po")
                    for i in range(HT8):
                        half = i % 2
                        nc.tensor.matmul(
                            po[half * B:(half + 1) * B, :], stT_t[i][:],
                            ok_t[i][:, ng * 500:(ng + 1) * 500],
                            start=(i < 2), stop=(i >= HT8 - 2),
                            tile_position=(0, half * B))
                    podd = sb.tile([B, 500], F32, tag="podd", bufs=3, name="podd")
                    nc.scalar.copy(podd[:], po[B:2 * B, :])
                    pr_sb = sb.tile([B, 500], F32, tag="prout", bufs=3, name="pr_sb")
                    nc.vector.tensor_add(pr_sb[:], po[0:B, :], podd[:])
                    nc.sync.dma_start(probs_sh[:, ng * 500:(ng + 1) * 500], pr_sb[:])
    nc.compile()
    return nc


_CACHE: dict = {}


def _get_nc():
    if "nc" not in _CACHE:
        _CACHE["nc"] = _build()
    return _CACHE["nc"]


def _prep_in_maps(inputs):
    f32 = np.float32
    dec = np.asarray(inputs["dec_input"])
    hid = np.asarray(inputs["hidden_state"], f32)
    enc = np.asarray(inputs["enc_output"], f32)
    emb = np.asarray(inputs["emb"], f32)
    W1 = np.asarray(inputs["W1_k"], f32)
    W1b = np.asarray(inputs["W1_b"], f32)
    W2 = np.asarray(inputs["W2_k"], f32)
    W2b = np.asarray(inputs["W2_b"], f32)
    Vk = np.asarray(inputs["V_k"], f32)
    gkf = np.asarray(inputs["gru_k"], f32)
    grkf = np.asarray(inputs["gru_rk"], f32)
    gb = np.asarray(inputs["gru_b"], f32)
    ok = np.asarray(inputs["out_k"], f32)
    ob = np.asarray(inputs["out_b"], f32)

    xemb = emb[np.asarray(dec[:, 0], dtype=np.int64)]          # (B, EMB)

    def bf(a):
        return np.ascontiguousarray(np.asarray(a, dtype=f32), dtype=NPBF16)

    def fc(a):
        return np.ascontiguousarray(a, dtype=f32)

    w2p = np.vstack([W2, (W1b + W2b)[None, :]])
    htp = np.vstack([hid.T, np.ones((1, B), f32)])
    vkr = Vk.reshape(H // 128, 128).T
    gkb = np.vstack([gkf, gb[0][None, :]])
    grkb = np.vstack([grkf, gb[1][None, :]])
    okp = ok
    ident = np.eye(128, dtype=NPBF16)

    htp_bf = bf(htp)
    vkr_bf = bf(vkr)
    w1_bf = bf(W1)
    w2p_bf = bf(w2p)
    xeT_bf = bf(xemb.T)

    maps = []
    for c in range(N_CORES):
        bs = slice(c * BC, (c + 1) * BC)
        hs = slice(c * HSH, (c + 1) * HSH)
        cols = np.concatenate([
            np.arange(c * HSH, (c + 1) * HSH),
            np.arange(H + c * HSH, H + (c + 1) * HSH),
            np.arange(2 * H + c * HSH, 2 * H + (c + 1) * HSH),
        ])
        encN_ = enc[bs].reshape(TOK, H)
        maps.append({
            "encT": bf(encN_.T),
            "encN": bf(encN_),
            "w1": w1_bf,
            "w2p": w2p_bf,
            "vkr": vkr_bf,
            "htp": htp_bf,
            "htl": np.ascontiguousarray(htp_bf[:, bs]),
            "gk": bf(gkb[:, cols]),
            "grk": bf(grkb[:, cols]),
            "xeT": xeT_bf,
            "hT": fc(hid[:, hs].T),
            "okp": bf(okp[:, c * VSH:(c + 1) * VSH]),
            "idbf": ident,
        })
    return maps


def _assemble(results, ob):
    probs = np.concatenate([results[c]["probs_sh"] for c in range(N_CORES)], axis=1)
    probs += ob[None, :]
    state = np.concatenate(
        [results[c]["state_shT"].T for c in range(N_CORES)], axis=1)
    attn = np.concatenate(
        [results[c]["attn_sh"] for c in range(N_CORES)], axis=0)[:, :, None]
    return (np.ascontiguousarray(probs), np.ascontiguousarray(state),
            np.ascontiguousarray(attn))


def run(inputs, trace=False, tmpdir=None, trace_cores=None):
    nc = _get_nc()
    in_maps = _prep_in_maps(inputs)
    res = run_bass_kernel_spmd(nc, in_maps, list(range(N_CORES)),
                               trace=trace, tmpdir=tmpdir,
                               trace_cores=trace_cores)
    ob = np.asarray(inputs["out_b"], np.float32)
    return _assemble(res.results, ob), res


def kernel(**inputs):
    (probs, state, attn), _ = run(inputs, trace=False)
    return probs, state, attn


# revision 13
# speedup vs baseline: 1.1259x; 1.1150x over previous
"""TRN2 Bass/Tile kernel: Bahdanau-attention GRU decoder step, 8-core SPMD.

Sharding:
  - Attention (the 17 GFLOP enc@W1 einsum) is data-parallel over batch
    (8 batches/core).
  - GRU is tensor-parallel over the hidden dim (128 units/core), computed in
    transposed [unit, batch] layout.
  - The 1024x32000 output projection is tensor-parallel over vocab
    (4000 cols/core).
  - Two tiny AllGathers stitch the stages: context^T (bf16, 16KB/rank) and
    state^T (bf16, 16KB/rank).

All heavy matmuls run in bf16 (1 cycle/row on the PE); accumulation is fp32
in PSUM. Biases are folded into appended weight rows on the host (the GRU
input bias rides the recurrent path's ones-row; V_b drops out of softmax).
"""
import numpy as np

import concourse.bacc as bacc
import concourse.bass as bass
import concourse.mybir as mybir
import concourse.tile as tile
from concourse.bass_utils import run_bass_kernel_spmd

N_CORES = 8
B, S, H, EMB, VOCAB = 64, 128, 1024, 512, 32000
BC = B // N_CORES        # batches per core
VSH = VOCAB // N_CORES   # vocab shard
HSH = H // N_CORES       # hidden shard
TOK = BC * S             # tokens per core (1024)
F32 = mybir.dt.float32
BF16 = mybir.dt.bfloat16
NPBF16 = mybir.dt.np(BF16)
AF = mybir.ActivationFunctionType
AX = mybir.AxisListType


def _build():
    nc = bacc.Bacc(None, num_devices=N_CORES)
    dd = nc.declare_dram_parameter

    # Per-core inputs (host-sharded / host-transposed / bias-folded).
    encT = dd("encT", [H, TOK], BF16, isOutput=False)        # [h, tok]
    encN = dd("encN", [TOK, H], BF16, isOutput=False)        # [tok, h]
    w1 = dd("w1", [H, H], BF16, isOutput=False)              # [h, k]
    w2p = dd("w2p", [H + 1, H], BF16, isOutput=False)        # [hin(+1), k]
    vkr = dd("vkr", [128, 8], BF16, isOutput=False)          # vkr[p,t]=V_k[t*128+p]
    htp = dd("htp", [H + 1, B], BF16, isOutput=False)        # hidden^T + ones row
    htl = dd("htl", [H + 1, BC], BF16, isOutput=False)       # local-batch slice of htp
    gk = dd("gk", [H + EMB + 1, 3 * HSH], BF16, isOutput=False)
    grk = dd("grk", [H + 1, 3 * HSH], BF16, isOutput=False)
    xeT = dd("xeT", [EMB, B], BF16, isOutput=False)          # x_emb^T
    hT = dd("hT", [HSH, B], F32, isOutput=False)             # hidden slice^T
    okp = dd("okp", [H, VSH], BF16, isOutput=False)          # out_k shard
    idbf = dd("idbf", [128, 128], BF16, isOutput=False)

    probs_sh = dd("probs_sh", [B, VSH], F32, isOutput=True)
    state_shT = dd("state_shT", [HSH, B], F32, isOutput=True)
    attn_sh = dd("attn_sh", [BC, S], F32, isOutput=True)

    HT8 = H // 128   # 8
    with tile.TileContext(nc) as tc:
        rings = [nc.sync, nc.sync]
        with (
            tc.tile_pool(name="sb", bufs=1) as sb,
            tc.tile_pool(name="sbw", bufs=1) as sbw,
            tc.tile_pool(name="dram", bufs=1, space="DRAM") as dram,
        ):
            # ---- input DMAs (emission order = DMA priority) ----
            encT_t = [sb.tile([128, TOK], BF16, name=f"encT{i}") for i in range(HT8)]
            w1_t = [sb.tile([128, H], BF16, name=f"w1_{i}") for i in range(HT8)]
            for i in range(HT8):
                for q in range(2):
                    r0, r1 = q * 64, (q + 1) * 64
                    rings[q].dma_start(encT_t[i][r0:r1, :],
                                       encT[i * 128 + r0:i * 128 + r1, :])
                    rings[1 - q].dma_start(w1_t[i][r0:r1, :],
                                           w1[i * 128 + r0:i * 128 + r1, :])
            w2_t = [sb.tile([128 if i < HT8 else 1, H], BF16, name=f"w2_{i}")
                    for i in range(HT8 + 1)]
            for i in range(HT8):
                for q in range(2):
                    r0, r1 = q * 64, (q + 1) * 64
                    rings[(i + q) % 2].dma_start(w2_t[i][r0:r1, :],
                                                 w2p[i * 128 + r0:i * 128 + r1, :])
            nc.sync.dma_start(w2_t[HT8][:], w2p[H:H + 1, :])
            htl_t = [sb.tile([128 if i < HT8 else 1, BC], BF16, name=f"htl{i}")
                     for i in range(HT8 + 1)]
            for i in range(HT8 + 1):
                rings[i % 2].dma_start(htl_t[i][:],
                                       htl[i * 128:min((i + 1) * 128, H + 1), :])
            vk_sb = sb.tile([128, 8], BF16)
            nc.sync.dma_start(vk_sb[:], vkr[:])
            id_sb = sb.tile([128, 128], BF16)
            nc.sync.dma_start(id_sb[:], idbf[:])
            encN_t = [sb.tile([128, H], BF16, name=f"encN{b}") for b in range(BC)]
            for b in range(BC):
                rings[b % 2].dma_start(encN_t[b][:], encN[b * S:(b + 1) * S, :])
            htp_t = [sb.tile([128 if i < HT8 else 1, B], BF16, name=f"htp{i}")
                     for i in range(HT8 + 1)]
            for i in range(HT8 + 1):
                rings[i % 2].dma_start(htp_t[i][:],
                                       htp[i * 128:min((i + 1) * 128, H + 1), :])
            ones_row = htp_t[HT8]  # [1, 64] of 1.0 (host-built)
            GKT = (H + EMB) // 128  # 12
            gk_t = [sb.tile([128 if i < GKT else 1, 3 * HSH], BF16, name=f"gk{i}")
                    for i in range(GKT + 1)]
            for i in range(GKT + 1):
                rings[i % 2].dma_start(gk_t[i][:],
                                       gk[i * 128:min((i + 1) * 128, H + EMB + 1), :])
            grk_t = [sb.tile([128 if i < HT8 else 1, 3 * HSH], BF16, name=f"grk{i}")
                     for i in range(HT8 + 1)]
            for i in range(HT8 + 1):
                rings[(i + 1) % 2].dma_start(grk_t[i][:],
                                             grk[i * 128:min((i + 1) * 128, H + 1), :])
            xe_t = [sb.tile([128, B], BF16, name=f"xe{i}") for i in range(EMB // 128)]
            for i in range(EMB // 128):
                rings[i % 2].dma_start(xe_t[i][:], xeT[i * 128:(i + 1) * 128, :])
            hT_sb = sb.tile([HSH, B], F32)
            nc.sync.dma_start(hT_sb[:], hT[:])
            # big output-projection weights last (prefetch, must not block the above)
            ok_t = [sbw.tile([128, VSH], BF16, name=f"ok{i}") for i in range(HT8)]
            for i in range(HT8):
                rings[i % 2].dma_start(ok_t[i][:], okp[i * 128:(i + 1) * 128, :])

            # collective bounce buffers
            cc1_in = dram.tile([BC, H], BF16)                 # context shard (b-major)
            cc1_out = dram.tile([B, H], BF16, addr_space="Shared")
            cc2_in = dram.tile([HSH, B], BF16)                # state^T shard
            cc2_out = dram.tile([N_CORES * HSH, B], BF16, addr_space="Shared")

            with (
                tc.tile_pool(name="pf", bufs=3, space="PSUM") as pf_pool,
                tc.tile_pool(name="psmall", bufs=2, space="PSUM") as psm_pool,
                tc.tile_pool(name="pscore", bufs=1, space="PSUM") as ps_pool,
            ):
                # ---- hW2^T[k, b_local] = W2p^T @ hiddenT(local) ----
                h2_sb = [sb.tile([128, BC], F32, name=f"h2_{k}") for k in range(HT8)]
                for kt in range(HT8):
                    ph2 = psm_pool.tile([128, BC], F32, tag="psm", name="ph2")
                    for i in range(HT8 + 1):
                        nc.tensor.matmul(
                            ph2[:], w2_t[i][:, kt * 128:(kt + 1) * 128], htl_t[i][:],
                            start=(i == 0), stop=(i == HT8))
                    nc.vector.tensor_copy(h2_sb[kt][:], ph2[:])

                # ---- feat^T = tanh(W1^T @ enc^T + hW2^T) ----
                featT_t = [sb.tile([128, TOK], BF16, name=f"featT{k}") for k in range(HT8)]
                for kt in range(HT8):
                    for ng in range(2):
                        pf = pf_pool.tile([128, 512], F32, tag="pf", name="pf")
                        for i in range(HT8):
                            nc.tensor.matmul(
                                pf[:], w1_t[i][:, kt * 128:(kt + 1) * 128],
                                encT_t[i][:, ng * 512:(ng + 1) * 512],
                                start=(i == 0), stop=(i == HT8 - 1))
                        for j in range(4):
                            b = ng * 4 + j
                            nc.scalar.activation(
                                featT_t[kt][:, b * 128:(b + 1) * 128],
                                pf[:, j * 128:(j + 1) * 128],
                                AF.Tanh, bias=h2_sb[kt][:, b:b + 1], scale=1.0)

                # ---- score[tok] = V^T @ feat^T ----
                score_row = sb.tile([1, TOK], F32)
                for ng in range(2):
                    ps = ps_pool.tile([1, 512], F32, tag="ps", name="ps")
                    for kt in range(HT8):
                        nc.tensor.matmul(
                            ps[:], vk_sb[:, kt:kt + 1],
                            featT_t[kt][:, ng * 512:(ng + 1) * 512],
                            start=(kt == 0), stop=(kt == HT8 - 1))
                    nc.vector.tensor_copy(score_row[:, ng * 512:(ng + 1) * 512], ps[:])

                # reshape [1, 1024] -> [8, 128] (SBUF->SBUF DMA)
                score_sb = sb.tile([BC, S], F32)
                nc.gpsimd.dma_start(score_sb[:], score_row[:])

                # ---- softmax over s per batch row ----
                negm = sb.tile([BC, 1], F32)
                nc.vector.reduce_max(negm[:], score_sb[:], axis=AX.X, negate=True)
                esum = sb.tile([BC, 1], F32)
                attn_e = sb.tile([BC, S], F32)
                nc.scalar.activation(attn_e[:], score_sb[:], AF.Exp,
                                     bias=negm[:], scale=1.0, accum_out=esum[:])
                rinv = sb.tile([BC, 1], F32)
                nc.vector.reciprocal(rinv[:], esum[:])
                attn_f = sb.tile([BC, S], F32)
                nc.vector.tensor_scalar_mul(attn_f[:], attn_e[:], rinv[:])
                nc.gpsimd.dma_start(attn_sh[:], attn_f[:])
                attn_bf = sb.tile([BC, S], BF16)
                nc.vector.tensor_scalar_mul(attn_bf[:], attn_e[:], rinv[:])

                # ---- attn^T [s, b] via PE transpose ----
                p_at = psm_pool.tile([S, BC], BF16, tag="psm", name="p_at")
                nc.tensor.transpose(p_at[:], attn_bf[:], id_sb[0:BC, 0:BC])
                attnT = sb.tile([S, BC], BF16)
                nc.vector.tensor_copy(attnT[:], p_at[:])

                # ---- context^T[h, b] = enc[b]^T @ attn[b], then PE-transpose
                # back to b-major so the AllGather buffer is contiguous ----
                ctxN_sb = sb.tile([BC, H], BF16)
                for i in range(HT8):
                    pc = psm_pool.tile([128, BC], F32, tag="psm", name="pc")
                    for b in range(BC):
                        nc.tensor.matmul(
                            pc[:, b:b + 1], encN_t[b][:, i * 128:(i + 1) * 128],
                            attnT[:, b:b + 1], start=True, stop=True)
                    ctxT_sb = sb.tile([128, BC], BF16, tag="ctxT", bufs=2,
                                      name="ctxT_sb")
                    nc.vector.tensor_copy(ctxT_sb[:], pc[:])
                    pcn = psm_pool.tile([BC, 128], BF16, tag="pcn", name="pcn")
                    nc.tensor.transpose(pcn[:], ctxT_sb[:], id_sb[:])
                    nc.vector.tensor_copy(ctxN_sb[:, i * 128:(i + 1) * 128], pcn[:])
                nc.gpsimd.dma_start(cc1_in[:], ctxN_sb[:])

            # ---- AllGather context^T ----
            nc.gpsimd.collective_compute(
                "AllGather", mybir.AluOpType.bypass,
                replica_groups=[list(range(N_CORES))],
                ins=[cc1_in.opt()], outs=[cc1_out.opt()])

            # xT tiles: [h, b_global] — gather columns across cores
            ctxg_sb = sb.tile([B, H], BF16)
            for q in range(4):
                r0, r1 = q * 16, (q + 1) * 16
                nc.sync.dma_start(ctxg_sb[r0:r1, :], cc1_out[r0:r1, :])

            with (
                tc.tile_pool(name="pg", bufs=1, space="PSUM") as pg_pool,
                tc.tile_pool(name="pout", bufs=2, space="PSUM") as po_pool,
            ):
                xT_t = [sb.tile([128, B], BF16, name=f"xT{i}") for i in range(HT8)]
                for i in range(HT8):
                    ptr = po_pool.tile([128, B], BF16, tag="po", name="ptr")
                    nc.tensor.transpose(ptr[:], ctxg_sb[:, i * 128:(i + 1) * 128],
                                        id_sb[0:B, 0:B])
                    nc.vector.tensor_copy(xT_t[i][:], ptr[:])
                # ---- GRU in transposed layout: [unit, b] ----
                phm = [pg_pool.tile([HSH, B], F32, tag=f"phm{g}", name=f"phm{g}") for g in range(3)]
                for g in range(3):
                    for i in range(HT8 + 1):
                        nc.tensor.matmul(
                            phm[g][:], grk_t[i][:, g * HSH:(g + 1) * HSH], htp_t[i][:],
                            start=(i == 0), stop=(i == HT8))
                pxm = [pg_pool.tile([HSH, B], F32, tag=f"pxm{g}", name=f"pxm{g}") for g in range(3)]
                for g in range(3):
                    for i in range(GKT + 1):
                        rhs = xT_t[i] if i < HT8 else (
                            xe_t[i - HT8] if i < GKT else ones_row)
                        nc.tensor.matmul(
                            pxm[g][:], gk_t[i][:, g * HSH:(g + 1) * HSH], rhs[:],
                            start=(i == 0), stop=(i == GKT))

                hm_sb = [sb.tile([HSH, B], F32, name=f"hm{g}") for g in range(3)]
                for g in range(3):
                    nc.vector.tensor_copy(hm_sb[g][:], phm[g][:])
                zpre = sb.tile([HSH, B], F32)
                nc.vector.tensor_add(zpre[:], pxm[0][:], hm_sb[0][:])
                z_sb = sb.tile([HSH, B], F32)
                nc.scalar.activation(z_sb[:], zpre[:], AF.Sigmoid)
                rpre = sb.tile([HSH, B], F32)
                nc.vector.tensor_add(rpre[:], pxm[1][:], hm_sb[1][:])
                r_sb = sb.tile([HSH, B], F32)
                nc.scalar.activation(r_sb[:], rpre[:], AF.Sigmoid)
                rhh = sb.tile([HSH, B], F32)
                nc.vector.tensor_mul(rhh[:], r_sb[:], hm_sb[2][:])
                hpre = sb.tile([HSH, B], F32)
                nc.vector.tensor_add(hpre[:], pxm[2][:], rhh[:])
                hc_sb = sb.tile([HSH, B], F32)
                nc.scalar.activation(hc_sb[:], hpre[:], AF.Tanh)
                # state = hc + z*(h_old - hc)
                dd_sb = sb.tile([HSH, B], F32)
                nc.vector.tensor_sub(dd_sb[:], hT_sb[:], hc_sb[:])
                zd_sb = sb.tile([HSH, B], F32)
                nc.vector.tensor_mul(zd_sb[:], z_sb[:], dd_sb[:])
                stT_f = sb.tile([HSH, B], F32)
                nc.vector.tensor_add(stT_f[:], hc_sb[:], zd_sb[:])
                nc.gpsimd.dma_start(state_shT[:], stT_f[:])
                stT_bf_loc = sb.tile([HSH, B], BF16)
                nc.vector.tensor_copy(stT_bf_loc[:], stT_f[:])
                nc.gpsimd.dma_start(cc2_in[:], stT_bf_loc[:])

                # ---- AllGather state^T ----
                nc.gpsimd.collective_compute(
                    "AllGather", mybir.AluOpType.bypass,
                    replica_groups=[list(range(N_CORES))],
                    ins=[cc2_in.opt()], outs=[cc2_out.opt()])
                stT_t = [sb.tile([128, B], BF16, name=f"stT{i}") for i in range(HT8)]
                for i in range(HT8):
                    nc.sync.dma_start(stT_t[i][:], cc2_out[i * 128:(i + 1) * 128, :])

                # ---- probs shard = state @ out_k(+bias row) ----
                NG = VSH // 500  # 8 groups of 500
                for ng in range(NG):
                    po = po_pool.tile([128, 500], F32, tag="po", name="po")
                    for i in range(HT8):
                        half = i % 2
                        nc.tensor.matmul(
                            po[half * B:(half + 1) * B, :], stT_t[i][:],
                            ok_t[i][:, ng * 500:(ng + 1) * 500],
                            start=(i < 2), stop=(i >= HT8 - 2),
                            tile_position=(0, half * B))
                    podd = sb.tile([B, 500], F32, tag="podd", bufs=3, name="podd")
                    nc.scalar.copy(podd[:], po[B:2 * B, :])
                    pr_sb = sb.tile([B, 500], F32, tag="prout", bufs=3, name="pr_sb")
                    nc.vector.tensor_add(pr_sb[:], po[0:B, :], podd[:])
                    nc.sync.dma_start(probs_sh[:, ng * 500:(ng + 1) * 500], pr_sb[:])
    nc.compile()
    return nc


_CACHE: dict = {}


def _get_nc():
    if "nc" not in _CACHE:
        _CACHE["nc"] = _build()
    return _CACHE["nc"]


def _prep_in_maps(inputs):
    f32 = np.float32
    dec = np.asarray(inputs["dec_input"])
    hid = np.asarray(inputs["hidden_state"], f32)
    enc = np.asarray(inputs["enc_output"], f32)
    emb = np.asarray(inputs["emb"], f32)
    W1 = np.asarray(inputs["W1_k"], f32)
    W1b = np.asarray(inputs["W1_b"], f32)
    W2 = np.asarray(inputs["W2_k"], f32)
    W2b = np.asarray(inputs["W2_b"], f32)
    Vk = np.asarray(inputs["V_k"], f32)
    gkf = np.asarray(inputs["gru_k"], f32)
    grkf = np.asarray(inputs["gru_rk"], f32)
    gb = np.asarray(inputs["gru_b"], f32)
    ok = np.asarray(inputs["out_k"], f32)
    ob = np.asarray(inputs["out_b"], f32)

    xemb = emb[np.asarray(dec[:, 0], dtype=np.int64)]          # (B, EMB)

    def bf(a):
        return np.ascontiguousarray(np.asarray(a, dtype=f32), dtype=NPBF16)

    def fc(a):
        return np.ascontiguousarray(a, dtype=f32)

    w2p = np.vstack([W2, (W1b + W2b)[None, :]])
    htp = np.vstack([hid.T, np.ones((1, B), f32)])
    vkr = Vk.reshape(H // 128, 128).T
    gkb = np.vstack([gkf, gb[0][None, :]])
    grkb = np.vstack([grkf, gb[1][None, :]])
    okp = ok
    ident = np.eye(128, dtype=NPBF16)

    htp_bf = bf(htp)
    vkr_bf = bf(vkr)
    w1_bf = bf(W1)
    w2p_bf = bf(w2p)
    xeT_bf = bf(xemb.T)

    maps = []
    for c in range(N_CORES):
        bs = slice(c * BC, (c + 1) * BC)
        hs = slice(c * HSH, (c + 1) * HSH)
        cols = np.concatenate([
            np.arange(c * HSH, (c + 1) * HSH),
            np.arange(H + c * HSH, H + (c + 1) * HSH),
            np.arange(2 * H + c * HSH, 2 * H + (c + 1) * HSH),
        ])
        encN_ = enc[bs].reshape(TOK, H)
        maps.append({
            "encT": bf(encN_.T),
            "encN": bf(encN_),
            "w1": w1_bf,
            "w2p": w2p_bf,
            "vkr": vkr_bf,
            "htp": htp_bf,
            "htl": np.ascontiguousarray(htp_bf[:, bs]),
            "gk": bf(gkb[:, cols]),
            "grk": bf(grkb[:, cols]),
            "xeT": xeT_bf,
            "hT": fc(hid[:, hs].T),
            "okp": bf(okp[:, c * VSH:(c + 1) * VSH]),
            "idbf": ident,
        })
    return maps


def _assemble(results, ob):
    probs = np.concatenate([results[c]["probs_sh"] for c in range(N_CORES)], axis=1)
    probs += ob[None, :]
    state = np.concatenate(
        [results[c]["state_shT"].T for c in range(N_CORES)], axis=1)
    attn = np.concatenate(
        [results[c]["attn_sh"] for c in range(N_CORES)], axis=0)[:, :, None]
    return (np.ascontiguousarray(probs), np.ascontiguousarray(state),
            np.ascontiguousarray(attn))


def run(inputs, trace=False, tmpdir=None, trace_cores=None):
    nc = _get_nc()
    in_maps = _prep_in_maps(inputs)
    res = run_bass_kernel_spmd(nc, in_maps, list(range(N_CORES)),
                               trace=trace, tmpdir=tmpdir,
                               trace_cores=trace_cores)
    ob = np.asarray(inputs["out_b"], np.float32)
    return _assemble(res.results, ob), res


def kernel(**inputs):
    (probs, state, attn), _ = run(inputs, trace=False)
    return probs, state, attn
